# revision 10
# baseline (speedup 1.0000x reference)
"""Multi-head causal attention (B=8, S=1024, C=1024, H=16, dk=dv=64) on 8 trn2 cores.

Sharding: data-parallel over batch. Each NeuronCore processes one batch element
end-to-end (projections + attention + output projection); no collectives.

Per-core layout:
  inputs (host-prepped): xq/xk/xv = X^T [C, S] bf16, packed weights
  wq/wk [C, H*DK] (wq pre-scaled by 1/sqrt(dk)), wv [C, H*DV], wo [H*DV, C],
  biases in per-partition / replicated layouts.

  QT = wq.T @ xq  -> [H*DK, S]   (head-major rows)
  KT = wk.T @ xk  -> [H*DK, S]
  V  = xv.T @ wv  -> [S, H*DV]   (+ appended ones column per head)
  per head h, q-chunk: St[t, q] = KT_h.T-contract -> exp -> mask ->
    O^T/r accumulated via matmul(lhsT=[V_h | 1], rhs=P)  (row 64 = softmax denom)
  Y = concat(O)^T-contract @ wo + bo -> [S, C] f32
"""

import math
import os
import sys

import numpy as np

try:
    import concourse.bass as bass
except ImportError:  # make concourse importable in a bare grading dir
    for _p in ("/opt/trn_rl_repo", os.path.expanduser("~/.axon_site/_ro/trn_rl_repo")):
        if os.path.isdir(_p) and _p not in sys.path:
            sys.path.insert(0, _p)
    import concourse.bass as bass

from contextlib import ExitStack

import ml_dtypes

import concourse.mybir as mybir
import concourse.tile as tile
from concourse import bacc
from concourse.bass_utils import run_bass_kernel_spmd

B, S, C = 8, 1024, 1024
H, DK, DV = 16, 64, 64
P = 128
NT = 8  # number of 128-tiles along S / C / H*DK
CH = 512  # free-dim chunk (one PSUM bank of fp32)
NCH = S // CH

FP = mybir.dt.float32
BF = mybir.dt.bfloat16
BF_NP = ml_dtypes.bfloat16
AFT = mybir.ActivationFunctionType
ALU = mybir.AluOpType


def build_nc() -> bass.Bass:
    nc = bacc.Bacc()

    xq = nc.dram_tensor("xq", [C, S], BF, kind="ExternalInput")
    xk = nc.dram_tensor("xk", [C, S], BF, kind="ExternalInput")
    xv = nc.dram_tensor("xv", [C, S], BF, kind="ExternalInput")
    wq = nc.dram_tensor("wq", [C, H * DK], BF, kind="ExternalInput")
    wk = nc.dram_tensor("wk", [C, H * DK], BF, kind="ExternalInput")
    wv = nc.dram_tensor("wv", [C, H * DV], BF, kind="ExternalInput")
    wo = nc.dram_tensor("wo", [H * DV, C], BF, kind="ExternalInput")
    bqd = nc.dram_tensor("bq", [P, NT], FP, kind="ExternalInput")
    bkd = nc.dram_tensor("bk", [P, NT], FP, kind="ExternalInput")
    bvd = nc.dram_tensor("bv", [P, H * DV], FP, kind="ExternalInput")
    bod = nc.dram_tensor("bo", [1, C], FP, kind="ExternalInput")
    y = nc.dram_tensor("y", [S, C], FP, kind="ExternalOutput")

    # constants baked into the NEFF, packed into one tensor (single DMA)
    # cols 0:128   = additive causal mask for [t,q] diag block (0 if t<=q else -big)
    # row 0, cols 128:192 = ones[1,64]   (denominator replicate lhsT)
    # row 0, cols 192:320 = ones[1,128]  (output-bias matmul lhsT)
    cblk_np = np.zeros((P, 320), np.float32)
    cblk_np[:, 0:P] = np.where(
        np.triu(np.ones((P, P), bool)), 0.0, -60000.0
    )
    cblk_np[0, P : P + DV] = 1.0
    cblk_np[0, P + DV : P + DV + P] = 1.0
    cblk_d = nc.inline_tensor(cblk_np, "cblk")

    xq_r = xq.rearrange("(ko p) s -> p ko s", p=P)
    xk_r = xk.rearrange("(ko p) s -> p ko s", p=P)
    xv_r = xv.rearrange("(ko p) s -> p ko s", p=P)
    wq_r = wq.rearrange("(ko p) m -> p ko m", p=P)
    wk_r = wk.rearrange("(ko p) m -> p ko m", p=P)
    wv_r = wv.rearrange("(ko p) m -> p ko m", p=P)
    wo_r = wo.rearrange("(ko p) c -> p ko c", p=P)
    y_r = y.rearrange("(mo p) c -> p mo c", p=P)

    with tile.TileContext(nc) as tc, ExitStack() as octx:
        const = octx.enter_context(tc.tile_pool(name="const", bufs=1))
        qk = octx.enter_context(tc.tile_pool(name="qk", bufs=1))
        opool = octx.enter_context(tc.tile_pool(name="oT", bufs=1))
        ppool = octx.enter_context(tc.tile_pool(name="p", bufs=4))
        small = octx.enter_context(tc.tile_pool(name="small", bufs=4))
        ypool = octx.enter_context(tc.tile_pool(name="y", bufs=3))

        cblk_sb = const.tile([P, 320], FP, tag="cblk")
        nc.sync.dma_start(cblk_sb, cblk_d[:])
        trineg_sb = cblk_sb[:, 0:P]
        ones64_sb = cblk_sb[0:1, P : P + DV]
        ones128_sb = cblk_sb[0:1, P + DV : P + DV + P]
        bq_sb = const.tile([P, NT], FP, tag="bq")
        nc.sync.dma_start(bq_sb, bqd[:])
        bk_sb = const.tile([P, NT], FP, tag="bk")
        nc.sync.dma_start(bk_sb, bkd[:])
        bv_sb = const.tile([P, H * DV], FP, tag="bv")
        nc.sync.dma_start(bv_sb, bvd[:])
        bo_sb = const.tile([1, C], FP, tag="bo")
        nc.sync.dma_start(bo_sb, bod[:])
        wo_sb = const.tile([P, NT, C], BF, tag="wo")
        nc.sync.dma_start(wo_sb, wo_r)

        qT_sb = qk.tile([P, NT, S], BF, tag="qT")
        kT_sb = qk.tile([P, NT, S], BF, tag="kT")
        v_sb = qk.tile([P, NT, H, DV + 1], BF, tag="v")
        oT_sb = opool.tile([P, NT, S], BF, tag="oT")

        nc.vector.memset(v_sb[:, :, :, DV], 1.0)

        # ---------------- projections ----------------
        with ExitStack() as ictx:
            wpool = ictx.enter_context(tc.tile_pool(name="wqkv", bufs=2))
            xpool = ictx.enter_context(tc.tile_pool(name="xin", bufs=2))
            psproj = ictx.enter_context(
                tc.tile_pool(name="psproj", bufs=8, space="PSUM")
            )

            # Q^T and K^T: out[hk, s] ; lhsT = w tile [c, hk], rhs = x^T [c, s]
            for x_r, w_r, b_sb, out_sb in (
                (xq_r, wq_r, bq_sb, qT_sb),
                (xk_r, wk_r, bk_sb, kT_sb),
            ):
                w_sb = wpool.tile([P, NT, H * DK], BF, tag="w", name="w_sb")
                nc.sync.dma_start(w_sb, w_r)
                x_sb = xpool.tile([P, NT, S], BF, tag="x", name="x_sb")
                nc.sync.dma_start(x_sb, x_r)
                for n in range(NCH):
                    psums = []
                    for kc in range(NT):
                        for m in range(NT):
                            if kc == 0:
                                psums.append(psproj.tile([P, CH], FP, tag="proj", name=f"proj_ps_{m}"))
                            nc.tensor.matmul(
                                psums[m],
                                w_sb[:, kc, m * P : (m + 1) * P],
                                x_sb[:, kc, n * CH : (n + 1) * CH],
                                start=(kc == 0),
                                stop=(kc == NT - 1),
                            )
                    for m in range(NT):
                        nc.scalar.activation(
                            out_sb[:, m, n * CH : (n + 1) * CH],
                            psums[m],
                            AFT.Identity,
                            bias=b_sb[:, m : m + 1],
                            scale=1.0,
                        )

            # V: out[s, hv] ; lhsT = x^T tile [c, s], rhs = wv [c, hv]
            wv_sb = wpool.tile([P, NT, H * DV], BF, tag="w", name="wv_sb")
            nc.sync.dma_start(wv_sb, wv_r)
            xv_sb = xpool.tile([P, NT, S], BF, tag="x", name="xv_sb")
            nc.sync.dma_start(xv_sb, xv_r)
            for n in range(NCH):
                psums = []
                for kc in range(NT):
                    for m in range(NT):
                        if kc == 0:
                            psums.append(psproj.tile([P, CH], FP, tag="proj", name=f"proj_ps_{m}"))
                        nc.tensor.matmul(
                            psums[m],
                            xv_sb[:, kc, m * P : (m + 1) * P],
                            wv_sb[:, kc, n * CH : (n + 1) * CH],
                            start=(kc == 0),
                            stop=(kc == NT - 1),
                        )
                for m in range(NT):
                    dst = v_sb[:, m, 8 * n : 8 * (n + 1), 0:DV]
                    nc.vector.tensor_tensor(
                        dst,
                        psums[m].rearrange("p (h v) -> p h v", v=DV),
                        bv_sb[:, n * CH : (n + 1) * CH].rearrange(
                            "p (h v) -> p h v", v=DV
                        ),
                        ALU.add,
                    )

        # ---------------- attention ----------------
        ps_st = octx.enter_context(tc.tile_pool(name="ps_st", bufs=2, space="PSUM"))
        ps_o = octx.enter_context(tc.tile_pool(name="ps_o", bufs=2, space="PSUM"))
        ps_misc = octx.enter_context(tc.tile_pool(name="ps_misc", bufs=2, space="PSUM"))

        for h in range(H):
            hp, hm = h // 2, (h % 2) * DK
            for jc in range(NCH):
                po = ps_o.tile([P, CH], FP, tag="o", name=f"po_{h}_{jc}")[: DV + 1]
                i_list = list(range(0, min(NT, 4 * jc + 4)))
                for idx, i in enumerate(i_list):
                    # valid q columns in this chunk start at the diagonal
                    off = max(0, i * P - jc * CH)
                    w = CH - off
                    pst = ps_st.tile([P, CH], FP, tag="st", name=f"st_{h}_{jc}_{i}")[
                        :, :w
                    ]
                    nc.tensor.matmul(
                        pst,
                        kT_sb[hm : hm + DK, hp, i * P : (i + 1) * P],
                        qT_sb[hm : hm + DK, hp, jc * CH + off : (jc + 1) * CH],
                        start=True,
                        stop=True,
                    )
                    if i * P >= jc * CH:
                        # diagonal block: additive causal mask on the first 128 cols
                        nc.vector.tensor_tensor(
                            pst[:, 0:P], pst[:, 0:P], trineg_sb, ALU.add
                        )
                    pch = ppool.tile([P, CH], BF, tag="p", name=f"p_{h}_{jc}_{i}")[
                        :, :w
                    ]
                    nc.scalar.activation(pch, pst, AFT.Exp)
                    nc.tensor.matmul(
                        po[:, off:],
                        v_sb[:, i, h, :],
                        pch,
                        start=(idx == 0),
                        stop=(idx == len(i_list) - 1),
                    )
                # normalize: row DV of po is the softmax denominator
                rrow = small.tile([1, CH], FP, tag="rrow")
                nc.vector.tensor_copy(out=rrow, in_=po[DV : DV + 1, :])
                prep = ps_misc.tile([P, CH], FP, tag="misc", name=f"prep_{h}_{jc}")[:DV]
                nc.tensor.matmul(prep, ones64_sb, rrow, start=True, stop=True)
                rrep = small.tile([DV, CH], FP, tag="rrep")
                nc.vector.reciprocal(rrep, prep)
                nc.vector.tensor_tensor(
                    oT_sb[hm : hm + DV, hp, jc * CH : (jc + 1) * CH],
                    po[:DV],
                    rrep,
                    ALU.mult,
                )

        # ---------------- output projection ----------------
        for m in range(NT):
            for n in range(NCH):
                py = ps_misc.tile([P, CH], FP, tag="misc", name=f"py_{m}_{n}")
                for kc in range(NT):
                    nc.tensor.matmul(
                        py,
                        oT_sb[:, kc, m * P : (m + 1) * P],
                        wo_sb[:, kc, n * CH : (n + 1) * CH],
                        start=(kc == 0),
                        stop=False,
                    )
                # + bias: rank-1 update ones[s] x bo[c']
                nc.tensor.matmul(
                    py,
                    ones128_sb,
                    bo_sb[:, n * CH : (n + 1) * CH],
                    start=False,
                    stop=True,
                )
                yt = ypool.tile([P, CH], FP, tag="y")
                nc.scalar.copy(yt, py)
                nc.sync.dma_start(y_r[:, m, n * CH : (n + 1) * CH], yt)

    nc.finalize()
    return nc


_NC_CACHE = None


def _get_nc() -> bass.Bass:
    global _NC_CACHE
    if _NC_CACHE is None:
        _NC_CACHE = build_nc()
    return _NC_CACHE


def prep_shared(Wq, bq, Wk, bk, Wv, bv, Wo, bo):
    """Host-side packing of weights/biases (shared by all cores)."""
    scale = 1.0 / math.sqrt(DK)
    Wq = np.asarray(Wq, np.float32)
    Wk = np.asarray(Wk, np.float32)
    Wv = np.asarray(Wv, np.float32)
    Wo = np.asarray(Wo, np.float32)
    out = {
        "wq": np.ascontiguousarray(
            (Wq.transpose(1, 0, 2).reshape(C, H * DK) * scale).astype(BF_NP)
        ),
        "wk": np.ascontiguousarray(
            Wk.transpose(1, 0, 2).reshape(C, H * DK).astype(BF_NP)
        ),
        "wv": np.ascontiguousarray(
            Wv.transpose(1, 0, 2).reshape(C, H * DV).astype(BF_NP)
        ),
        "wo": Wo.astype(BF_NP),
        "bq": np.ascontiguousarray(
            (np.asarray(bq, np.float32).reshape(H * DK) * scale)
            .reshape(NT, P)
            .T.astype(np.float32)
        ),
        "bk": np.ascontiguousarray(
            np.asarray(bk, np.float32).reshape(NT, P).T.astype(np.float32)
        ),
        "bv": np.ascontiguousarray(
            np.broadcast_to(
                np.asarray(bv, np.float32).reshape(1, H * DV), (P, H * DV)
            ).astype(np.float32)
        ),
        "bo": np.ascontiguousarray(np.asarray(bo, np.float32).reshape(1, C)),
    }
    return out


def prep_core(q_embs_b, k_embs_b, v_embs_b):
    return {
        "xq": np.ascontiguousarray(np.asarray(q_embs_b, np.float32).T.astype(BF_NP)),
        "xk": np.ascontiguousarray(np.asarray(k_embs_b, np.float32).T.astype(BF_NP)),
        "xv": np.ascontiguousarray(np.asarray(v_embs_b, np.float32).T.astype(BF_NP)),
    }


def kernel(q_embs, k_embs, v_embs, Wq, bq, Wk, bk, Wv, bv, Wo, bo, **run_kwargs):
    nc = _get_nc()
    shared = prep_shared(Wq, bq, Wk, bk, Wv, bv, Wo, bo)
    q_embs = np.asarray(q_embs, np.float32)
    k_embs = np.asarray(k_embs, np.float32)
    v_embs = np.asarray(v_embs, np.float32)
    in_maps = []
    for b in range(B):
        m = dict(shared)
        m.update(prep_core(q_embs[b], k_embs[b], v_embs[b]))
        in_maps.append(m)
    res = run_bass_kernel_spmd(nc, in_maps, core_ids=list(range(B)), **run_kwargs)
    out = np.stack([res.results[i]["y"] for i in range(B)], axis=0)
    if run_kwargs:
        kernel.last_results = res
    return out


if __name__ == "__main__":
    rng = np.random.default_rng(0)
    inputs = {
        "q_embs": rng.standard_normal((B, S, C), np.float32),
        "k_embs": rng.standard_normal((B, S, C), np.float32),
        "v_embs": rng.standard_normal((B, S, C), np.float32),
        "Wq": rng.standard_normal((H, C, DK), np.float32) * 0.02,
        "bq": np.zeros((H, DK), np.float32),
        "Wk": rng.standard_normal((H, C, DK), np.float32) * 0.02,
        "bk": np.zeros((H, DK), np.float32),
        "Wv": rng.standard_normal((H, C, DV), np.float32) * 0.02,
        "bv": np.zeros((H, DV), np.float32),
        "Wo": rng.standard_normal((H * DV, C), np.float32) * 0.02,
        "bo": np.zeros((C,), np.float32),
    }
    out = kernel(**inputs)
    print(out.shape, out.dtype)


# revision 13
# speedup vs baseline: 1.2258x; 1.2258x over previous
"""Multi-head causal attention (B=8, S=1024, C=1024, H=16, dk=dv=64) on 8 trn2 cores.

Sharding: data-parallel over batch. Each NeuronCore processes one batch element
end-to-end (projections + attention + output projection); no collectives.

Per-core layout:
  inputs (host-prepped): xq/xk/xv = X^T [C, S] bf16, packed weights
  wq/wk [C, H*DK] (wq pre-scaled by 1/sqrt(dk)), wv [C, H*DV], wo [H*DV, C],
  biases in per-partition / replicated layouts.

  QT = wq.T @ xq  -> [H*DK, S]   (head-major rows)
  KT = wk.T @ xk  -> [H*DK, S]
  V  = xv.T @ wv  -> [S, H*DV]   (+ appended ones column per head)
  per head h, q-chunk: St[t, q] = KT_h.T-contract -> exp -> mask ->
    O^T/r accumulated via matmul(lhsT=[V_h | 1], rhs=P)  (row 64 = softmax denom)
  Y = concat(O)^T-contract @ wo + bo -> [S, C] f32
"""

import math
import os
import sys

import numpy as np

try:
    import concourse.bass as bass
except ImportError:  # make concourse importable in a bare grading dir
    for _p in ("/opt/trn_rl_repo", os.path.expanduser("~/.axon_site/_ro/trn_rl_repo")):
        if os.path.isdir(_p) and _p not in sys.path:
            sys.path.insert(0, _p)
    import concourse.bass as bass

from contextlib import ExitStack

import ml_dtypes

import concourse.mybir as mybir
import concourse.tile as tile
from concourse import bacc
from concourse.bass_utils import run_bass_kernel_spmd

B, S, C = 8, 1024, 1024
H, DK, DV = 16, 64, 64
P = 128
NT = 8  # number of 128-tiles along S / C / H*DK
CH = 512  # free-dim chunk (one PSUM bank of fp32)
NCH = S // CH

PAIR_HEADS = os.environ.get("K_PAIR", "1") == "1"

FP = mybir.dt.float32
BF = mybir.dt.bfloat16
BF_NP = ml_dtypes.bfloat16
AFT = mybir.ActivationFunctionType
ALU = mybir.AluOpType


def build_nc() -> bass.Bass:
    nc = bacc.Bacc()

    xq = nc.dram_tensor("xq", [C, S], BF, kind="ExternalInput")
    xk = nc.dram_tensor("xk", [C, S], BF, kind="ExternalInput")
    xv = nc.dram_tensor("xv", [C, S], BF, kind="ExternalInput")
    wq = nc.dram_tensor("wq", [C, H * DK], BF, kind="ExternalInput")
    wk = nc.dram_tensor("wk", [C, H * DK], BF, kind="ExternalInput")
    wv = nc.dram_tensor("wv", [C, H * DV], BF, kind="ExternalInput")
    wo = nc.dram_tensor("wo", [H * DV, C], BF, kind="ExternalInput")
    bqd = nc.dram_tensor("bq", [P, NT], FP, kind="ExternalInput")
    bkd = nc.dram_tensor("bk", [P, NT], FP, kind="ExternalInput")
    bvd = nc.dram_tensor("bv", [P, H * DV], FP, kind="ExternalInput")
    bod = nc.dram_tensor("bo", [1, C], FP, kind="ExternalInput")
    y = nc.dram_tensor("y", [S, C], FP, kind="ExternalOutput")

    # constants baked into the NEFF, packed into one tensor (single DMA)
    # cols 0:128   = additive causal mask for [t,q] diag block (0 if t<=q else -big)
    # row 0, cols 128:192 = ones[1,64]   (denominator replicate lhsT)
    # row 0, cols 192:320 = ones[1,128]  (output-bias matmul lhsT)
    cblk_np = np.zeros((P, 384), np.float32)
    cblk_np[:, 0:P] = np.where(
        np.triu(np.ones((P, P), bool)), 0.0, -60000.0
    )
    cblk_np[0, P : P + DV] = 1.0
    cblk_np[0, P + DV : P + DV + P] = 1.0
    cblk_np[0, P + DV + P : P + 2 * DV + P] = -1.0
    cblk_d = nc.inline_tensor(cblk_np, "cblk")

    xq_r = xq.rearrange("(ko p) s -> p ko s", p=P)
    xk_r = xk.rearrange("(ko p) s -> p ko s", p=P)
    xv_r = xv.rearrange("(ko p) s -> p ko s", p=P)
    wq_r = wq.rearrange("(ko p) m -> p ko m", p=P)
    wk_r = wk.rearrange("(ko p) m -> p ko m", p=P)
    wv_r = wv.rearrange("(ko p) m -> p ko m", p=P)
    wo_r = wo.rearrange("(ko p) c -> p ko c", p=P)
    y_r = y.rearrange("(mo p) c -> p mo c", p=P)

    with tile.TileContext(nc) as tc, ExitStack() as octx:
        const = octx.enter_context(tc.tile_pool(name="const", bufs=1))
        qk = octx.enter_context(tc.tile_pool(name="qk", bufs=1))
        opool = octx.enter_context(tc.tile_pool(name="oT", bufs=1))
        ppool = octx.enter_context(tc.tile_pool(name="p", bufs=4))
        small = octx.enter_context(tc.tile_pool(name="small", bufs=4))
        ypool = octx.enter_context(tc.tile_pool(name="y", bufs=3))

        cblk_sb = const.tile([P, 384], FP, tag="cblk")
        nc.sync.dma_start(cblk_sb, cblk_d[:])
        trineg_sb = cblk_sb[:, 0:P]
        ones64_sb = cblk_sb[0:1, P : P + DV]
        ones128_sb = cblk_sb[0:1, P + DV : P + DV + P]
        negones64_sb = cblk_sb[0:1, P + DV + P : P + 2 * DV + P]
        bq_sb = const.tile([P, NT], FP, tag="bq")
        nc.sync.dma_start(bq_sb, bqd[:])
        bk_sb = const.tile([P, NT], FP, tag="bk")
        nc.sync.dma_start(bk_sb, bkd[:])
        bv_sb = const.tile([P, H * DV], FP, tag="bv")
        nc.sync.dma_start(bv_sb, bvd[:])
        bo_sb = const.tile([1, C], FP, tag="bo")
        nc.sync.dma_start(bo_sb, bod[:])
        wo_sb = const.tile([P, NT, C], BF, tag="wo")
        nc.sync.dma_start(wo_sb, wo_r)

        qT_sb = qk.tile([P, NT, S], BF, tag="qT")
        kT_sb = qk.tile([P, NT, S], BF, tag="kT")
        v_sb = qk.tile([P, NT, H, DV + 1], BF, tag="v")
        oT_sb = opool.tile([P, NT, S], BF, tag="oT")

        nc.vector.memset(v_sb[:, :, :, DV], 1.0)

        # ---------------- projections ----------------
        with ExitStack() as ictx:
            wpool = ictx.enter_context(tc.tile_pool(name="wqkv", bufs=2))
            xpool = ictx.enter_context(tc.tile_pool(name="xin", bufs=2))
            psproj = ictx.enter_context(
                tc.tile_pool(name="psproj", bufs=8, space="PSUM")
            )

            # Q^T and K^T: out[hk, s] ; lhsT = w tile [c, hk], rhs = x^T [c, s]
            for x_r, w_r, b_sb, out_sb in (
                (xq_r, wq_r, bq_sb, qT_sb),
                (xk_r, wk_r, bk_sb, kT_sb),
            ):
                w_sb = wpool.tile([P, NT, H * DK], BF, tag="w", name="w_sb")
                nc.sync.dma_start(w_sb, w_r)
                x_sb = xpool.tile([P, NT, S], BF, tag="x", name="x_sb")
                nc.sync.dma_start(x_sb, x_r)
                for n in range(NCH):
                    psums = []
                    for kc in range(NT):
                        for m in range(NT):
                            if kc == 0:
                                psums.append(psproj.tile([P, CH], FP, tag="proj", name=f"proj_ps_{m}"))
                            nc.tensor.matmul(
                                psums[m],
                                w_sb[:, kc, m * P : (m + 1) * P],
                                x_sb[:, kc, n * CH : (n + 1) * CH],
                                start=(kc == 0),
                                stop=(kc == NT - 1),
                            )
                    for m in range(NT):
                        nc.scalar.activation(
                            out_sb[:, m, n * CH : (n + 1) * CH],
                            psums[m],
                            AFT.Identity,
                            bias=b_sb[:, m : m + 1],
                            scale=1.0,
                        )

            # V: out[s, hv] ; lhsT = x^T tile [c, s], rhs = wv [c, hv]
            wv_sb = wpool.tile([P, NT, H * DV], BF, tag="w", name="wv_sb")
            nc.sync.dma_start(wv_sb, wv_r)
            xv_sb = xpool.tile([P, NT, S], BF, tag="x", name="xv_sb")
            nc.sync.dma_start(xv_sb, xv_r)
            for n in range(NCH):
                psums = []
                for kc in range(NT):
                    for m in range(NT):
                        if kc == 0:
                            psums.append(psproj.tile([P, CH], FP, tag="proj", name=f"proj_ps_{m}"))
                        nc.tensor.matmul(
                            psums[m],
                            xv_sb[:, kc, m * P : (m + 1) * P],
                            wv_sb[:, kc, n * CH : (n + 1) * CH],
                            start=(kc == 0),
                            stop=(kc == NT - 1),
                        )
                for m in range(NT):
                    dst = v_sb[:, m, 8 * n : 8 * (n + 1), 0:DV]
                    nc.vector.tensor_tensor(
                        dst,
                        psums[m].rearrange("p (h v) -> p h v", v=DV),
                        bv_sb[:, n * CH : (n + 1) * CH].rearrange(
                            "p (h v) -> p h v", v=DV
                        ),
                        ALU.add,
                    )

        # ---------------- attention ----------------
        # Head pairs (2h, 2h+1) sit on partitions 0:64 / 64:128 of the same
        # qT/kT tile, so their K=64 St matmuls land on disjoint PE row-groups
        # and run concurrently when issued back-to-back.
        ps_st = octx.enter_context(tc.tile_pool(name="ps_st", bufs=3, space="PSUM"))
        ps_o = octx.enter_context(tc.tile_pool(name="ps_o", bufs=3, space="PSUM"))
        ps_misc = octx.enter_context(tc.tile_pool(name="ps_misc", bufs=2, space="PSUM"))

        subs = range(2) if PAIR_HEADS else range(1)

        def attn_group(hp, jc, heads):
            pos = {}
            i_list = list(range(0, min(NT, 4 * jc + 4)))
            for sub in heads:
                pos[sub] = ps_o.tile(
                    [P, CH], FP, tag="o", name=f"po_{hp}_{jc}_{sub}"
                )[: DV + 1]
            for idx, i in enumerate(i_list):
                # valid q columns in this chunk start at the diagonal
                off = max(0, i * P - jc * CH)
                w = CH - off
                pchs = {}
                for sub in heads:
                    hm = sub * DK
                    pst = ps_st.tile(
                        [P, CH], FP, tag="st", name=f"st_{hp}_{jc}_{i}_{sub}"
                    )[:, :w]
                    nc.tensor.matmul(
                        pst,
                        kT_sb[hm : hm + DK, hp, i * P : (i + 1) * P],
                        qT_sb[hm : hm + DK, hp, jc * CH + off : (jc + 1) * CH],
                        start=True,
                        stop=True,
                    )
                    if i * P >= jc * CH:
                        # diagonal block: additive causal mask on first 128 cols
                        nc.vector.tensor_tensor(
                            pst[:, 0:P], pst[:, 0:P], trineg_sb, ALU.add
                        )
                    pch = ppool.tile(
                        [P, CH], BF, tag="p", name=f"p_{hp}_{jc}_{i}_{sub}"
                    )[:, :w]
                    nc.scalar.activation(pch, pst, AFT.Exp)
                    pchs[sub] = pch
                for sub in heads:
                    nc.tensor.matmul(
                        pos[sub][:, off:],
                        v_sb[:, i, 2 * hp + sub, :],
                        pchs[sub],
                        start=(idx == 0),
                        stop=(idx == len(i_list) - 1),
                    )
            for sub in heads:
                hm = sub * DK
                po = pos[sub]
                # row DV of po is the softmax denominator r; apply 1/r as
                # exp(-ln r): Ln on ACT, PE rank-1 replicate with -1 row,
                # Exp on ACT -> rrep [64, q], then one DVE multiply.
                rln = small.tile([1, CH], FP, tag="rln")
                nc.scalar.activation(rln, po[DV : DV + 1, :], AFT.Ln)
                prep = ps_misc.tile(
                    [P, CH], FP, tag="misc", name=f"prep_{hp}_{jc}_{sub}"
                )[:DV]
                nc.tensor.matmul(prep, negones64_sb, rln, start=True, stop=True)
                rrep = small.tile([DV, CH], FP, tag="rrep")
                nc.scalar.activation(rrep, prep, AFT.Exp)
                nc.vector.tensor_tensor(
                    oT_sb[hm : hm + DV, hp, jc * CH : (jc + 1) * CH],
                    po[:DV],
                    rrep,
                    ALU.mult,
                )

        for hp in range(H // 2):
            for jc in range(NCH):
                if PAIR_HEADS:
                    attn_group(hp, jc, [0, 1])
                else:
                    attn_group(hp, jc, [0])
                    attn_group(hp, jc, [1])

        # ---------------- output projection ----------------
        for m in range(NT):
            for n in range(NCH):
                py = ps_misc.tile([P, CH], FP, tag="misc", name=f"py_{m}_{n}")
                for kc in range(NT):
                    nc.tensor.matmul(
                        py,
                        oT_sb[:, kc, m * P : (m + 1) * P],
                        wo_sb[:, kc, n * CH : (n + 1) * CH],
                        start=(kc == 0),
                        stop=False,
                    )
                # + bias: rank-1 update ones[s] x bo[c']
                nc.tensor.matmul(
                    py,
                    ones128_sb,
                    bo_sb[:, n * CH : (n + 1) * CH],
                    start=False,
                    stop=True,
                )
                yt = ypool.tile([P, CH], FP, tag="y")
                nc.scalar.copy(yt, py)
                nc.sync.dma_start(y_r[:, m, n * CH : (n + 1) * CH], yt)

    nc.finalize()
    return nc


_NC_CACHE = None


def _get_nc() -> bass.Bass:
    global _NC_CACHE
    if _NC_CACHE is None:
        _NC_CACHE = build_nc()
    return _NC_CACHE


def prep_shared(Wq, bq, Wk, bk, Wv, bv, Wo, bo):
    """Host-side packing of weights/biases (shared by all cores)."""
    scale = 1.0 / math.sqrt(DK)
    Wq = np.asarray(Wq, np.float32)
    Wk = np.asarray(Wk, np.float32)
    Wv = np.asarray(Wv, np.float32)
    Wo = np.asarray(Wo, np.float32)
    out = {
        "wq": np.ascontiguousarray(
            (Wq.transpose(1, 0, 2).reshape(C, H * DK) * scale).astype(BF_NP)
        ),
        "wk": np.ascontiguousarray(
            Wk.transpose(1, 0, 2).reshape(C, H * DK).astype(BF_NP)
        ),
        "wv": np.ascontiguousarray(
            Wv.transpose(1, 0, 2).reshape(C, H * DV).astype(BF_NP)
        ),
        "wo": Wo.astype(BF_NP),
        "bq": np.ascontiguousarray(
            (np.asarray(bq, np.float32).reshape(H * DK) * scale)
            .reshape(NT, P)
            .T.astype(np.float32)
        ),
        "bk": np.ascontiguousarray(
            np.asarray(bk, np.float32).reshape(NT, P).T.astype(np.float32)
        ),
        "bv": np.ascontiguousarray(
            np.broadcast_to(
                np.asarray(bv, np.float32).reshape(1, H * DV), (P, H * DV)
            ).astype(np.float32)
        ),
        "bo": np.ascontiguousarray(np.asarray(bo, np.float32).reshape(1, C)),
    }
    return out


def prep_core(q_embs_b, k_embs_b, v_embs_b):
    return {
        "xq": np.ascontiguousarray(np.asarray(q_embs_b, np.float32).T.astype(BF_NP)),
        "xk": np.ascontiguousarray(np.asarray(k_embs_b, np.float32).T.astype(BF_NP)),
        "xv": np.ascontiguousarray(np.asarray(v_embs_b, np.float32).T.astype(BF_NP)),
    }


def kernel(q_embs, k_embs, v_embs, Wq, bq, Wk, bk, Wv, bv, Wo, bo, **run_kwargs):
    nc = _get_nc()
    shared = prep_shared(Wq, bq, Wk, bk, Wv, bv, Wo, bo)
    q_embs = np.asarray(q_embs, np.float32)
    k_embs = np.asarray(k_embs, np.float32)
    v_embs = np.asarray(v_embs, np.float32)
    in_maps = []
    for b in range(B):
        m = dict(shared)
        m.update(prep_core(q_embs[b], k_embs[b], v_embs[b]))
        in_maps.append(m)
    res = run_bass_kernel_spmd(nc, in_maps, core_ids=list(range(B)), **run_kwargs)
    out = np.stack([res.results[i]["y"] for i in range(B)], axis=0)
    if run_kwargs:
        kernel.last_results = res
    return out


if __name__ == "__main__":
    rng = np.random.default_rng(0)
    inputs = {
        "q_embs": rng.standard_normal((B, S, C), np.float32),
        "k_embs": rng.standard_normal((B, S, C), np.float32),
        "v_embs": rng.standard_normal((B, S, C), np.float32),
        "Wq": rng.standard_normal((H, C, DK), np.float32) * 0.02,
        "bq": np.zeros((H, DK), np.float32),
        "Wk": rng.standard_normal((H, C, DK), np.float32) * 0.02,
        "bk": np.zeros((H, DK), np.float32),
        "Wv": rng.standard_normal((H, C, DV), np.float32) * 0.02,
        "bv": np.zeros((H, DV), np.float32),
        "Wo": rng.standard_normal((H * DV, C), np.float32) * 0.02,
        "bo": np.zeros((C,), np.float32),
    }
    out = kernel(**inputs)
    print(out.shape, out.dtype)
